# revision 8
# baseline (speedup 1.0000x reference)
"""Trainium2 Bass kernel: CaputoFractionalActivation (tanh base, alpha=0.5, 10 terms).

Math: the reference evaluates tanh at 11 points x - k*h (h in [1e-6, 1e-3]) and
takes the Caputo finite-difference series.  Because h is tiny, the series
collapses (Taylor expansion around x, with S0 = sum_j w_j = 0 exactly) to

    out = t - (1 - t^2) * (S1 + S2 * h * t) + O(h^2)

with t = tanh(x), h = beta*x + gamma affine in x via the global min/max,
S1 = -sum(COEF), S2 = -sum((2k+1)*COEF).  The truncation error (~5e-5 absmax)
is far below the f32 reference's own rounding noise (~8e-3 absmax vs f64).

Distribution: data-parallel shard of x across 8 cores; each core reduces a
stride-8 subsample to [max, -min] partials, gpsimd partition_all_reduce makes
rows uniform, an 8-core AllReduce(max) on 8 floats produces the global values,
then the purely-elementwise pipeline runs from SBUF-resident tensors.
"""

import math

import numpy as np

import concourse.bacc as bacc
import concourse.mybir as mybir
from concourse import bass_isa, tile
from concourse.bass_utils import run_bass_kernel_spmd

N_CORES = 8
ALPHA = 0.5
N_TERMS = 10
MIN_STEP = 1e-6
MAX_STEP = 1e-3
_COEF = [
    ((-1.0) ** k) * math.gamma(ALPHA + k + 1.0) / (math.factorial(k) * math.gamma(ALPHA + 1.0))
    for k in range(N_TERMS)
]
S1 = -sum(_COEF)
S2 = -sum((2 * k + 1) * c for k, c in enumerate(_COEF))
C0 = 1.0 / (2.0 * S1)

# Full input (4, 4096, 1024) f32, sharded 8 ways on axis 1 -> (4, 512, 1024)
# per core = 2,097,152 elements = [128 partitions, 16384 free].
B, T, D = 4, 4096, 1024
P = 128
F = (B * T * D) // (N_CORES * P)  # 16384


def emit(nc, x_d, o_d, F, FD, sfx=""):
    """Emit the per-core program. x_d/o_d: [P, F] f32 DRAM APs."""
    with tile.TileContext(nc) as tc:
        emit_in_tc(tc, x_d, o_d, F, FD, sfx=sfx)


def emit_in_tc(tc, x_d, o_d, F, FD, sfx=""):
    nc = tc.nc
    f32 = mybir.dt.float32
    bf16 = mybir.dt.bfloat16
    AT = mybir.AluOpType
    AF = mybir.ActivationFunctionType
    nt = F // FD
    zc = -(S1 * C0 * C0 + S1)  # z = S1*(t+C0)^2 + zc = t + S1*t^2 - S1

    if True:
        with (
            tc.tile_pool(name="resx" + sfx, bufs=1) as px,
            tc.tile_pool(name="resz" + sfx, bufs=1) as pz,
            tc.tile_pool(name="resw" + sfx, bufs=1) as pw,
            tc.tile_pool(name="tmpf" + sfx, bufs=4) as pf,
            tc.tile_pool(name="tmpb" + sfx, bufs=4) as pb,
            tc.tile_pool(name="smal" + sfx, bufs=1) as ps,
            tc.tile_pool(name="dram" + sfx, bufs=1, space="DRAM") as pd,
        ):
            x_sb = px.tile([P, F], f32, tag="x")
            z_sb = pz.tile([P, F], bf16, tag="z")
            w_sb = pw.tile([P, F], bf16, tag="w")
            c0b = ps.tile([P, 1], f32, tag="s_c0")
            nc.vector.memset(c0b[:], C0)

            # ---- phase A: load, tanh, z = t + S1 t^2 - S1, W' = (t^2-1)*t ----
            for i in range(nt):
                sl = slice(i * FD, (i + 1) * FD)
                nc.sync.dma_start(x_sb[:, sl], x_d[:, sl])
                t = pf.tile([P, FD], f32, tag="tmpf")
                nc.scalar.activation(t[:], x_sb[:, sl], AF.Tanh)
                y = pf.tile([P, FD], f32, tag="tmpf")
                nc.scalar.activation(y[:], t[:], AF.Square, bias=c0b[:])
                nc.vector.tensor_scalar(z_sb[:, sl], y[:], S1, zc, AT.mult, AT.add)
                tb = pb.tile([P, FD], bf16, tag="tmpb")
                nc.vector.tensor_copy(tb[:], t[:])
                qb = pb.tile([P, FD], bf16, tag="tmpb")
                nc.vector.tensor_tensor(qb[:], tb[:], tb[:], AT.mult)
                nc.vector.scalar_tensor_tensor(
                    w_sb[:, sl], qb[:], 1.0, tb[:], AT.subtract, AT.mult
                )

            # ---- phase B: global min/max -> b1 = S2*beta, b2 = S2*gamma ----
            mx = ps.tile([P, 1], f32, tag="s_mx")
            mn = ps.tile([P, 1], f32, tag="s_mn")
            sub = x_sb[:, ::8]  # stride-8 subsample; error in h is ~1e-5 relative
            nc.vector.tensor_reduce(mx[:], sub, mybir.AxisListType.X, AT.max)
            nc.vector.tensor_reduce(mn[:], sub, mybir.AxisListType.X, AT.min)
            pk = ps.tile([P, 2], f32, tag="s_pk")
            nc.vector.tensor_copy(pk[:, 0:1], mx[:])
            nc.vector.tensor_scalar(pk[:, 1:2], mn[:], -1.0, None, AT.mult)
            pr = ps.tile([P, 2], f32, tag="s_pr")
            nc.gpsimd.partition_all_reduce(pr[:], pk[:], 128, bass_isa.ReduceOp.max)
            cin = pd.tile([P, 2], f32, tag="d_in")
            cout = pd.tile([P, 2], f32, tag="d_out")
            nc.gpsimd.dma_start(cin[:], pr[:])
            nc.gpsimd.collective_compute(
                "AllReduce",
                AT.max,
                replica_groups=[list(range(N_CORES))],
                ins=[cin[:].opt()],
                outs=[cout[:].opt()],
            )
            gl = ps.tile([P, 2], f32, tag="s_gl")
            nc.gpsimd.dma_start(gl[:], cout[:])
            # gl[:,0] = gmax, gl[:,1] = -gmin  (identical on every partition)
            rng = ps.tile([P, 1], f32, tag="s_rng")
            nc.vector.tensor_tensor(rng[:], gl[:, 0:1], gl[:, 1:2], AT.add)
            inv = ps.tile([P, 1], f32, tag="s_inv")
            nc.vector.reciprocal(inv[:], rng[:])
            b1 = ps.tile([P, 1], f32, tag="s_b1")
            nc.vector.tensor_scalar(b1[:], inv[:], S2 * (MAX_STEP - MIN_STEP), None, AT.mult)
            tmp = ps.tile([P, 1], f32, tag="s_tmp")
            nc.vector.tensor_tensor(tmp[:], gl[:, 1:2], b1[:], AT.mult)
            b2 = ps.tile([P, 1], f32, tag="s_b2")
            nc.vector.tensor_scalar(b2[:], tmp[:], S2 * MIN_STEP, None, AT.add)

            # ---- phase C: hh = b1*x + b2 (= S2*h), out = z + hh*W' ----
            for i in range(nt):
                sl = slice(i * FD, (i + 1) * FD)
                hb = pb.tile([P, FD], bf16, tag="tmpb")
                nc.vector.tensor_scalar(hb[:], x_sb[:, sl], b1[:], b2[:], AT.mult, AT.add)
                mb = pb.tile([P, FD], bf16, tag="tmpb")
                nc.vector.tensor_tensor(mb[:], hb[:], w_sb[:, sl], AT.mult)
                o = pf.tile([P, FD], f32, tag="tmpf")
                nc.vector.tensor_tensor(o[:], mb[:], z_sb[:, sl], AT.add)
                nc.sync.dma_start(o_d[:, sl], o[:])


def build(F=F, FD=2048, reps=1):
    nc = bacc.Bacc("TRN2", target_bir_lowering=False, debug=False, num_devices=N_CORES)
    f32 = mybir.dt.float32
    x_d = nc.dram_tensor("x", [P, F], f32, kind="ExternalInput").ap()
    o_d = nc.dram_tensor("out", [P, F], f32, kind="ExternalOutput").ap()
    if reps == 0:
        # near-empty program for launch-overhead calibration
        with tile.TileContext(nc) as tc:
            with tc.tile_pool(name="cal", bufs=1) as pc:
                tcal = pc.tile([1, 2], f32, tag="cal")
                nc.sync.dma_start(tcal[:], x_d[:1, :2])
                nc.sync.dma_start(o_d[:1, :2], tcal[:])
    for r in range(reps):
        emit(nc, x_d, o_d, F, FD, sfx=f"_r{r}")
    nc.compile()
    return nc


_NC_CACHE = {}


def run(x, trace=False, **kw):
    """x: full (4, 4096, 1024) f32. Returns (full_out, BassKernelResults)."""
    key = "nc"
    if key not in _NC_CACHE:
        _NC_CACHE[key] = build()
    nc = _NC_CACHE[key]
    ts = T // N_CORES
    in_maps = [
        {"x": np.ascontiguousarray(x[:, i * ts : (i + 1) * ts, :]).reshape(P, F)}
        for i in range(N_CORES)
    ]
    br = run_bass_kernel_spmd(nc, in_maps, core_ids=list(range(N_CORES)), trace=trace, **kw)
    shards = [br.results[i]["out"].reshape(B, ts, D) for i in range(N_CORES)]
    out = np.concatenate(shards, axis=1)
    return out, br


def kernel(**inputs):
    x = np.asarray(inputs["x"], dtype=np.float32)
    out, _ = run(x)
    return out.astype(np.float32)
